# revision 27
# baseline (speedup 1.0000x reference)
"""ClusterNet (vq_codebook) Trainium2 kernel — two collective-free launches.

Computes, for z (8192, 256) and centroids (64, 256):
  sim  = euclidean_dist(z, centroids)                  (8192, 64)
  Q    = rownorm(1 / (1 + sim))
  P    = rownorm(Q^2 / colsum(Q))
and returns (Q, P), matching the reference nn_ClusterNet module.

Distribution: data-parallel over the batch across 8 NeuronCores (1024
rows/core), centroids replicated.  The global column-sum of Q is reduced
on the host between two launches — the on-device AllReduce costs
~95us/exec (NRT cc-op rendezvous; measured) and shared DRAM is only
visible within core pairs, so a second launch (~15us fixed cost) is the
cheapest global reduction.

Launch A (per core): z chunks are DMA'd on parallel queues; bf16 casts
on DVE; all transposes are single-instruction XBAR DMA transposes
(dma_start_transpose) instead of PE transpose + PSUM copyback chains.
dist^2 = (-2 z.c^T + |c|^2) accumulates in PSUM (one region-wide rank-1
per chunk for |c|^2, whose row vector comes straight from a ones-matmul
over cT^2); |z|^2 is computed on the otherwise-idle GPSIMD engine and
enters as the per-partition bias of the per-tile ACT sqrt.  A dummy
sqrt hoists the ACT table load before the data arrives.  U = 1/(1+sim)
via DVE fast reciprocal, row-normalize to Q, f32 ones-matmuls give
per-tile column sums (host folds).

Launch B (per core): host sends ssb = sqrt(1/colsum); one f32 PE matmul
broadcasts it to 128 partitions; then v = Q*ssb, P = rownorm(v^2) on DVE.
"""

import os
import sys

if "/opt/trn_rl_repo" not in sys.path:
    sys.path.insert(0, "/opt/trn_rl_repo")

import numpy as np

import concourse.bass as bass
import concourse.bacc as bacc
import concourse.tile as tile
from concourse import mybir
from concourse.masks import make_identity

NCORES = 8
BS = 1024          # rows per core
T = 8              # 128-row tiles per core
TG = 2             # tiles per DMA/transpose chunk
NG = T // TG       # chunks
HT = T // 2        # tiles per half (elementwise-chain granularity)
H = 256            # feature dim
K = 64             # clusters
F32 = mybir.dt.float32
BF16 = mybir.dt.bfloat16
AF = mybir.ActivationFunctionType


def build_kernel_a():
    nc = bacc.Bacc("TRN2", target_bir_lowering=False, debug=False,
                   num_devices=NCORES)
    z_d = nc.dram_tensor("z", [BS, H], F32, kind="ExternalInput")
    c_d = nc.dram_tensor("centroids", [K, H], F32, kind="ExternalInput")
    q_d = nc.dram_tensor("qout", [BS, K], F32, kind="ExternalOutput")
    cs_d = nc.dram_tensor("cs", [K], F32, kind="ExternalOutput")

    with tile.TileContext(nc) as tc:
        with (
            tc.tile_pool(name="consts", bufs=1) as consts,
            tc.tile_pool(name="sb", bufs=1) as sb,
            tc.tile_pool(name="ptz", bufs=2, space="PSUM") as ptz,
            tc.tile_pool(name="psum", bufs=1, space="PSUM") as psum,
        ):
            ones_row_bf = consts.tile([1, 128], BF16)
            nc.vector.memset(ones_row_bf, 1.0)
            ones_col_bf = consts.tile([128, 1], BF16)
            nc.vector.memset(ones_col_bf, 1.0)
            ones_col_f = consts.tile([128, 1], F32)
            nc.vector.memset(ones_col_f, 1.0)
            ident_bf = consts.tile([128, 128], BF16)
            make_identity(nc, ident_bf)

            # hoist the sqrt ACT table load before any data dependency
            dummy = sb.tile([1, 1], F32)
            nc.scalar.activation(dummy, ones_col_f[0:1, 0:1], AF.Sqrt)

            # ---- input DMAs: z chunks + centroids on parallel queues ----
            z_nat = sb.tile([128, T, H], F32)
            z_t = z_d[:].rearrange("(t p) h -> t p h", p=128)
            c_nat = sb.tile([K, H], F32)
            nc.scalar.dma_start(out=c_nat, in_=c_d[:])
            dma_eng = [nc.sync, nc.scalar, nc.gpsimd, nc.sync]
            for g in range(NG):
                t0 = g * TG
                dma_eng[g].dma_start(
                    out=z_nat[:, t0 : t0 + TG, :],
                    in_=z_t[t0 : t0 + TG].rearrange("t p h -> p t h"),
                )

            # cast every chunk up front so no cast is stranded behind
            # later DVE work (the casts gate transposes and matmuls);
            # chunks 2-3 cast on ACT, whose early window is idle
            z_bf = sb.tile([128, T, H], BF16)
            for g in range(NG):
                t0 = g * TG
                eng = nc.vector if g < 2 else nc.scalar
                if g < 2:
                    eng.tensor_copy(z_bf[:, t0 : t0 + TG, :],
                                    z_nat[:, t0 : t0 + TG, :])
                else:
                    nc.scalar.copy(z_bf[:, t0 : t0 + TG, :],
                                   z_nat[:, t0 : t0 + TG, :])

            # ---- centroids: (-2 c)^T via PE transpose; cn2 row via
            # ones-matmul over (cT2)^2/4 ----
            c_bf = sb.tile([K, H], BF16)
            nc.scalar.copy(c_bf, c_nat)
            c2_bf = sb.tile([K, H], BF16)
            nc.scalar.activation(c2_bf, c_nat, AF.Copy, bias=0.0, scale=-2.0)
            pct = psum.tile([128, 2, K], BF16)
            for j in range(2):
                nc.tensor.transpose(
                    pct[:, j, :], c2_bf[:, j * 128 : (j + 1) * 128],
                    ident_bf[0:K, 0:K],
                )
            cT2 = sb.tile([128, 2, K], BF16)
            nc.vector.tensor_copy(cT2, pct)
            cT2sq = sb.tile([128, 2, K], BF16)
            nc.vector.tensor_tensor(out=cT2sq, in0=cT2, in1=cT2,
                                    op=mybir.AluOpType.mult)
            # cn2 accumulates in the (later-used) colsum bank to stay
            # within the 8-bank PSUM budget
            csP = psum.tile([1, T * K], F32)
            for j in range(2):
                nc.tensor.matmul(csP[0:1, 0:K], ones_col_bf, cT2sq[:, j, :],
                                 start=(j == 0), stop=(j == 1))
            cn2row_bf = sb.tile([1, K], BF16)
            nc.scalar.activation(cn2row_bf, csP[0:1, 0:K], AF.Copy, bias=0.0,
                                 scale=0.25)

            # ---- per chunk: zT via PE transpose (chunks 0-1) or XBAR
            # DMA transpose (chunks 2-3), zn2 = rowsum(z^2) (GPSIMD mult
            # + DVE reduce), dist matmuls (PE) ----
            z2 = sb.tile([128, T, H], BF16)
            zn2 = sb.tile([128, T], F32)
            zT = sb.tile([128, T, 2, 128], BF16)
            pd = [psum.tile([128, TG, K], F32, name=f"pd{g}")
                  for g in range(NG)]
            for g in range(NG):
                t0 = g * TG
                if g < 2:
                    pzt = ptz.tile([128, 2 * TG, 128], BF16, tag="zt")
                    for tt in range(TG):
                        t = t0 + tt
                        for j in range(2):
                            nc.tensor.transpose(
                                pzt[:, 2 * tt + j, :],
                                z_bf[:, t, j * 128 : (j + 1) * 128],
                                ident_bf,
                            )
                    nc.vector.tensor_copy(
                        zT[:, t0 : t0 + TG, :, :],
                        pzt[:].rearrange("p (t j) h -> p t j h", j=2))
                else:
                    nc.sync.dma_start_transpose(
                        out=zT[:, t0 : t0 + TG, :, :],
                        in_=z_bf[:, t0 : t0 + TG, :],
                    )
                nc.gpsimd.tensor_tensor(
                    out=z2[:, t0 : t0 + TG, :], in0=z_bf[:, t0 : t0 + TG, :],
                    in1=z_bf[:, t0 : t0 + TG, :], op=mybir.AluOpType.mult)
                nc.vector.reduce_sum(zn2[:, t0 : t0 + TG],
                                     z2[:, t0 : t0 + TG, :],
                                     axis=mybir.AxisListType.X)
                # dist^2 partial: rank-1 |c|^2 over the chunk, then
                # per-tile -2 z.c matmuls accumulated on top
                nc.tensor.matmul(
                    pd[g][:, :, :],
                    ones_row_bf[0:1, :],
                    cn2row_bf[:, None, :].to_broadcast((1, TG, K)),
                    start=True, stop=False,
                )
                for tt in range(TG):
                    t = t0 + tt
                    nc.tensor.matmul(pd[g][:, tt, :], zT[:, t, 0, :],
                                     cT2[:, 0, :], start=False, stop=False)
                    nc.tensor.matmul(pd[g][:, tt, :], zT[:, t, 1, :],
                                     cT2[:, 1, :], start=False, stop=True)

            # ---- per half: sim = Sqrt(pd + zn2), u1 = 1 + sim (ACT),
            # U = 1/u1, Q = rownorm(U) (DVE), colsum (PE bf16) ----
            sim = sb.tile([128, T, K], F32)
            u1 = sb.tile([128, T, K], F32)
            u = sb.tile([128, T, K], F32)
            u_bf = sb.tile([128, T, K], BF16)
            rU = sb.tile([128, T], F32)
            rUi = sb.tile([128, T], F32)
            rUi_bf = sb.tile([128, T], BF16)
            q_sb = sb.tile([128, T, K], F32)
            q_out = q_d[:].rearrange("(t p) k -> p t k", p=128)
            for g in range(NG):
                sl = slice(g * TG, (g + 1) * TG)
                for tt in range(TG):
                    t = g * TG + tt
                    nc.scalar.activation(sim[:, t, :], pd[g][:, tt, :],
                                         AF.Sqrt, bias=zn2[:, t : t + 1])
                    nc.scalar.activation(u1[:, t, :], sim[:, t, :],
                                         AF.Identity, bias=1.0)
                nc.vector.reciprocal_approx_fast(
                    out=u[:, sl, :].rearrange("p t k -> p (t k)"),
                    in_=u1[:, sl, :].rearrange("p t k -> p (t k)"))
                nc.vector.reduce_sum(rU[:, sl], u[:, sl, :],
                                     axis=mybir.AxisListType.X)
                nc.vector.reciprocal(rUi[:, sl], rU[:, sl])
                # colsum(Q) = rUi^T @ U in bf16 (independent roundings
                # average out over the 1024-row sum) — final [1, K],
                # ready as soon as U/rUi are, not gated on Q
                nc.vector.tensor_copy(u_bf[:, sl, :], u[:, sl, :])
                nc.vector.tensor_copy(rUi_bf[:, sl], rUi[:, sl])
                for tt in range(TG):
                    t = g * TG + tt
                    nc.tensor.matmul(csP[0:1, K : 2 * K],
                                     rUi_bf[:, t : t + 1], u_bf[:, t, :],
                                     start=(t == 0), stop=(t == T - 1))
                nc.vector.tensor_tensor(
                    out=q_sb[:, sl, :],
                    in0=u[:, sl, :],
                    in1=rUi[:, sl, None].to_broadcast((128, TG, K)),
                    op=mybir.AluOpType.mult,
                )
                if g % 2 == 1:
                    hs = slice((g - 1) * TG, (g + 1) * TG)
                    nc.sync.dma_start(out=q_out[:, hs, :],
                                      in_=q_sb[:, hs, :])
            cs_sb = sb.tile([1, K], F32)
            nc.vector.tensor_copy(cs_sb, csP[0:1, K : 2 * K])
            nc.sync.dma_start(out=cs_d[:], in_=cs_sb)

    nc.compile()
    return nc


def build_kernel_b():
    nc = bacc.Bacc("TRN2", target_bir_lowering=False, debug=False,
                   num_devices=NCORES)
    q_d = nc.dram_tensor("q", [BS, K], F32, kind="ExternalInput")
    ssb_d = nc.dram_tensor("ssb", [K], F32, kind="ExternalInput")
    p_d = nc.dram_tensor("pout", [BS, K], F32, kind="ExternalOutput")

    with tile.TileContext(nc) as tc:
        with (
            tc.tile_pool(name="consts", bufs=1) as consts,
            tc.tile_pool(name="sb", bufs=1) as sb,
            tc.tile_pool(name="psum", bufs=1, space="PSUM") as psum,
        ):
            q_sb = sb.tile([128, T, K], F32)
            q_t = q_d[:].rearrange("(t p) k -> p t k", p=128)
            nc.sync.dma_start(out=q_sb[:, 0:HT, :], in_=q_t[:, 0:HT, :])
            nc.scalar.dma_start(out=q_sb[:, HT:T, :], in_=q_t[:, HT:T, :])
            ss_sb = sb.tile([1, K], F32)
            nc.gpsimd.dma_start(
                out=ss_sb,
                in_=bass.AP(tensor=ssb_d[:].tensor, offset=0,
                            ap=[[0, 1], [1, K]]),
            )
            ones_row_f = consts.tile([1, 128], F32)
            nc.vector.memset(ones_row_f, 1.0)
            ssP = psum.tile([128, K], F32)
            nc.tensor.matmul(ssP, ones_row_f, ss_sb, start=True, stop=True)

            v = sb.tile([128, T, K], F32)
            v2 = sb.tile([128, T, K], F32)
            rP = sb.tile([128, T], F32)
            rPi = sb.tile([128, T], F32)
            p_sb = sb.tile([128, T, K], F32)
            p_t = p_d[:].rearrange("(t p) k -> p t k", p=128)
            for g in range(NG):
                sl = slice(g * TG, (g + 1) * TG)
                nc.vector.tensor_tensor(
                    out=v[:, sl, :], in0=q_sb[:, sl, :],
                    in1=ssP[:, None, :].to_broadcast((128, TG, K)),
                    op=mybir.AluOpType.mult)
                nc.vector.tensor_tensor(out=v2[:, sl, :], in0=v[:, sl, :],
                                        in1=v[:, sl, :],
                                        op=mybir.AluOpType.mult)
                nc.vector.reduce_sum(rP[:, sl], v2[:, sl, :],
                                     axis=mybir.AxisListType.X)
                nc.vector.reciprocal(rPi[:, sl], rP[:, sl])
                nc.vector.tensor_tensor(
                    out=p_sb[:, sl, :], in0=v2[:, sl, :],
                    in1=rPi[:, sl, None].to_broadcast((128, TG, K)),
                    op=mybir.AluOpType.mult)
                if g % 2 == 1:
                    hs = slice((g - 1) * TG, (g + 1) * TG)
                    nc.sync.dma_start(out=p_t[:, hs, :], in_=p_sb[:, hs, :])

    nc.compile()
    return nc


_NC_CACHE = {}


def _get_nc(which):
    if which not in _NC_CACHE:
        _NC_CACHE[which] = (build_kernel_a if which == "a" else build_kernel_b)()
    return _NC_CACHE[which]


def kernel(z: np.ndarray, centroids: np.ndarray):
    from concourse.bass_utils import run_bass_kernel_spmd

    z = np.ascontiguousarray(np.asarray(z, dtype=np.float32))
    centroids = np.ascontiguousarray(np.asarray(centroids, dtype=np.float32))
    assert z.shape == (NCORES * BS, H) and centroids.shape == (K, H)

    nc_a = _get_nc("a")
    in_a = [{"z": z[c * BS : (c + 1) * BS], "centroids": centroids}
            for c in range(NCORES)]
    res_a = run_bass_kernel_spmd(nc_a, in_a, core_ids=list(range(NCORES)))
    Q = np.concatenate([res_a.results[c]["qout"] for c in range(NCORES)], 0)
    s = np.sum([res_a.results[c]["cs"] for c in range(NCORES)], axis=0)
    ssb = np.sqrt(1.0 / s).astype(np.float32)

    nc_b = _get_nc("b")
    in_b = [{"q": np.ascontiguousarray(Q[c * BS : (c + 1) * BS]), "ssb": ssb}
            for c in range(NCORES)]
    res_b = run_bass_kernel_spmd(nc_b, in_b, core_ids=list(range(NCORES)))
    P = np.concatenate([res_b.results[c]["pout"] for c in range(NCORES)], 0)
    return (Q, P)
